# revision 1
# baseline (speedup 1.0000x reference)
"""GQA multi-head attention (B=2, S=2048, HID=4096, 32 q-heads / 8 kv-heads,
tanh soft-cap, causal) on 8 TRN2 NeuronCores.

Sharding: tensor-parallel over heads. Core c owns kv-head c and q-heads
4c..4c+3 (Wq/Wk/Wv column slices, Wo row slice). Each core computes a partial
output out_c^T; the host sums the 8 partials and transposes back.

Layout strategy on-core: all activations kept transposed (feature-major:
partition = feature, free = token).
  QT[d, t] = Wq^T X^T      (moving operand = X^T chunks, stationary = Wq tiles)
  KT[d, t] = Wk^T X^T
  V [t, d]                 (via PE transpose of the VT projection)
  S^T[k, q] = KT_tile-as-stationary @ QT            (one matmul per k-tile)
  P^T = exp(30*tanh(S^T * mult/30)) * causal_mask   (ScalarE; capped scores
                                                     need no max subtraction)
  rowsum bcast = allones^T @ P^T                    (PE, fused reduce+bcast)
  O'^T[d, q] = V_tile-as-stationary @ P^T           (accumulated over k-tiles)
  A^T = O'^T * 1/rowsum                             (DVE, evict to bf16)
  out^T[hid, t] = Wo_tile-as-stationary @ A^T       (partial, bf16 to HBM)

Wo output tiles are deferred and re-emitted interleaved into the next token
chunk's attention (and the next batch's projections) so the PE always has
independent matmul work while PSUM-eviction round-trips drain.
"""

import sys

if "/opt/trn_rl_repo" not in sys.path:
    sys.path.insert(0, "/opt/trn_rl_repo")

import numpy as np
import ml_dtypes

BF = ml_dtypes.bfloat16

HID = 4096
S = 2048
B = 2
D = 128          # head dim
NHL = 4          # local q heads per core
CW = NHL * D     # 512, local q-proj width / wo row count
TOKCH = 128      # token chunk for projections
NCH = S // TOKCH
QCH = 512        # query chunk for attention
NQC = S // QCH
NKT = S // 128   # k-tiles per batch
NDT = HID // 128
ATTN_MULT = 0.08838834764831845
CAP = 30.0

_CACHED = {}

DEF_CFG = dict(
    sc_bufs=2, ov_bufs=1, bc_merged=False, bc_bufs=1, mm_bufs=1, wo_bufs=1,
    tanh=True, rowsum="pe",
)


def _build(reps=1, cfg=None):
    cfg = dict(DEF_CFG, **(cfg or {}))
    import concourse.mybir as mybir
    import concourse.tile as tile
    from concourse import bacc
    from concourse.masks import make_identity

    bf16 = mybir.dt.bfloat16
    f32 = mybir.dt.float32

    nc = bacc.Bacc(num_devices=8)
    xt_d = nc.dram_tensor("xt", [B, HID, S], bf16, kind="ExternalInput")
    wq_d = nc.dram_tensor("wq", [HID, CW], bf16, kind="ExternalInput")
    wk_d = nc.dram_tensor("wk", [HID, D], bf16, kind="ExternalInput")
    wv_d = nc.dram_tensor("wv", [HID, D], bf16, kind="ExternalInput")
    wo_d = nc.dram_tensor("wo", [CW, HID], bf16, kind="ExternalInput")
    msk_d = nc.dram_tensor("msk", [128, 4, QCH], bf16, kind="ExternalInput")
    out_d = nc.dram_tensor("out_t", [B, HID, S], bf16, kind="ExternalOutput")

    Tanh = mybir.ActivationFunctionType.Tanh
    Exp = mybir.ActivationFunctionType.Exp

    with tile.TileContext(nc) as tc:
        with (
            tc.tile_pool(name="consts", bufs=1) as consts,
            tc.tile_pool(name="weights", bufs=1) as wpool,
            tc.tile_pool(name="xin", bufs=2) as xpool,
            tc.tile_pool(name="qkv", bufs=2) as qkvpool,
            tc.tile_pool(name="atp", bufs=1) as atpool,
            tc.tile_pool(name="es", bufs=2) as espool,
            tc.tile_pool(name="rcp", bufs=2) as rcppool,
            tc.tile_pool(name="accp", bufs=2) as accpool,
            tc.tile_pool(name="vst", bufs=2) as vstpool,
            tc.tile_pool(name="osta", bufs=2) as outpool,
            tc.tile_pool(name="ps_sc", bufs=cfg["sc_bufs"], space="PSUM") as ps_sc,
            tc.tile_pool(name="ps_ov", bufs=cfg["ov_bufs"], space="PSUM") as ps_ov,
            tc.tile_pool(name="ps_bc", bufs=cfg["bc_bufs"], space="PSUM") as ps_bc,
            tc.tile_pool(name="ps_mm", bufs=cfg["mm_bufs"], space="PSUM") as ps_mm,
            tc.tile_pool(name="ps_wo", bufs=cfg["wo_bufs"] or 1, space="PSUM") as ps_wo,
        ):
            # --- persistent weights/constants in SBUF ---
            def load_w(dram, free, tag):
                t = wpool.tile([128, NDT, free], bf16, tag=tag)
                nc.sync.dma_start(
                    t[:], dram.ap().rearrange("(po pi) f -> pi po f", pi=128)
                )
                return t

            wq_sb = load_w(wq_d, CW, "wq")
            wk_sb = load_w(wk_d, D, "wk")
            wv_sb = load_w(wv_d, D, "wv")
            wo_sb = wpool.tile([128, CW // 128, HID], bf16)
            nc.sync.dma_start(
                wo_sb[:], wo_d.ap().rearrange("(po pi) f -> pi po f", pi=128)
            )
            msk_sb = consts.tile([128, 4, QCH], bf16)
            nc.sync.dma_start(msk_sb[:], msk_d.ap())
            ones_bf = consts.tile([128, 128], bf16)
            nc.vector.memset(ones_bf[:], 1.0)
            ones_f32 = consts.tile([128, 128], f32)
            nc.vector.memset(ones_f32[:], 1.0)
            ident = consts.tile([128, 128], bf16)
            make_identity(nc, ident[:])

            xt_r = xt_d.ap().rearrange("b (po pi) t -> pi b po t", pi=128)

            wo_tag = "wo" if cfg["wo_bufs"] else "mm"
            wo_pool = ps_wo if cfg["wo_bufs"] else ps_mm
            wo_jobs = []

            def emit_wo(n):
                # drain up to n deferred Wo output-tile jobs; interleaving
                # these among attention/projection work keeps the PE fed
                # while PSUM eviction round-trips drain
                for _ in range(min(n, len(wo_jobs))):
                    jb, jat, jq0, ht = wo_jobs.pop(0)
                    po = wo_pool.tile([128, QCH], f32, tag=wo_tag)
                    for ct in range(CW // 128):
                        nc.tensor.matmul(
                            po[:],
                            wo_sb[:, ct, ht * 128 : (ht + 1) * 128],
                            jat[:, ct, jq0 : jq0 + QCH],
                            start=(ct == 0),
                            stop=(ct == CW // 128 - 1),
                        )
                    ost = outpool.tile([128, QCH], bf16)
                    if ht % 2 == 0:
                        nc.scalar.copy(ost[:], po[:])
                    else:
                        nc.vector.tensor_copy(ost[:], po[:])
                    nc.sync.dma_start(
                        out_d.ap()[jb, ht * 128 : (ht + 1) * 128, jq0 : jq0 + QCH],
                        ost[:],
                    )

            def proj_group(w_sb, hsl, xt_sb, out_ap):
                p = ps_mm.tile([128, TOKCH], f32, tag="mm")
                for dt in range(NDT):
                    nc.tensor.matmul(
                        p[:],
                        w_sb[:, dt, hsl],
                        xt_sb[:, dt, :],
                        start=(dt == 0),
                        stop=(dt == NDT - 1),
                    )
                nc.vector.tensor_copy(out_ap, p[:])

            def proj_chunk(b, c, xt_sb, qt_sb, kt_sb, v_sb):
                t0 = c * TOKCH
                nc.sync.dma_start(xt_sb[:], xt_r[:, b, :, t0 : t0 + TOKCH])
                for h in range(NHL):
                    proj_group(
                        wq_sb, slice(h * 128, (h + 1) * 128), xt_sb,
                        qt_sb[:, h, t0 : t0 + TOKCH],
                    )
                proj_group(wk_sb, slice(0, D), xt_sb, kt_sb[:, t0 : t0 + TOKCH])
                vt_sb = vstpool.tile([128, TOKCH], bf16)
                p = ps_mm.tile([128, TOKCH], f32, tag="mm")
                for dt in range(NDT):
                    nc.tensor.matmul(
                        p[:], wv_sb[:, dt, :], xt_sb[:, dt, :],
                        start=(dt == 0), stop=(dt == NDT - 1),
                    )
                nc.vector.tensor_copy(vt_sb[:], p[:])
                for i in range(TOKCH // 128):
                    tt = c * (TOKCH // 128) + i
                    ptp = ps_mm.tile([128, 128], bf16, tag="mm")
                    nc.tensor.transpose(
                        ptp[:], vt_sb[:, i * 128 : (i + 1) * 128], ident[:]
                    )
                    nc.vector.tensor_copy(v_sb[:, tt, :], ptp[:])

            def scores_exp(es, qc, h, nkt, qt_sb, kt_sb):
                q0 = qc * QCH
                for p in range(nkt // 2):
                    ps = ps_sc.tile([128, 2, QCH], f32, tag="sc")
                    for i in range(2):
                        kt = 2 * p + i
                        nc.tensor.matmul(
                            ps[:, i, :],
                            kt_sb[:, kt * 128 : (kt + 1) * 128],
                            qt_sb[:, h, q0 : q0 + QCH],
                            start=True,
                            stop=True,
                        )
                    if cfg["tanh"]:
                        nc.scalar.activation(
                            ps[:], ps[:], Tanh, scale=ATTN_MULT / CAP
                        )
                        nc.scalar.activation(
                            es[:, 2 * p : 2 * p + 2, :], ps[:], Exp, scale=CAP
                        )
                    else:
                        nc.scalar.activation(
                            es[:, 2 * p : 2 * p + 2, :], ps[:], Exp,
                            scale=ATTN_MULT,
                        )
                for j in range(4):
                    kt = 4 * qc + j
                    nc.vector.tensor_mul(
                        es[:, kt, :], es[:, kt, :], msk_sb[:, j, :]
                    )

            def rowsum_bcast(es, nkt, bc_ap):
                if cfg["rowsum"] == "dve":
                    acc = accpool.tile([128, QCH], f32, tag="acc")
                    nc.vector.tensor_add(acc[:], es[:, 0, :], es[:, 1, :])
                    for kt in range(2, nkt):
                        nc.vector.tensor_add(acc[:], acc[:], es[:, kt, :])
                    nc.tensor.matmul(
                        bc_ap, ones_f32[:], acc[:], start=True, stop=True
                    )
                else:
                    for kt in range(nkt):
                        nc.tensor.matmul(
                            bc_ap,
                            ones_bf[:],
                            es[:, kt, :],
                            start=(kt == 0),
                            stop=(kt == nkt - 1),
                        )

            def attn_unit(b, qc, h, qt_sb, kt_sb, v_sb, at_sb):
                q0 = qc * QCH
                nkt = 4 * qc + 4
                es = espool.tile([128, nkt, QCH], bf16, tag="es")
                scores_exp(es, qc, h, nkt, qt_sb, kt_sb)
                if cfg["bc_merged"]:
                    ovbc = ps_ov.tile([128, 2, QCH], f32, tag="ov")
                    ov_ap, bc_ap = ovbc[:, 0, :], ovbc[:, 1, :]
                else:
                    ov_t = ps_ov.tile([128, QCH], f32, tag="ov")
                    if cfg.get("bc_in_mm"):
                        bc_t = ps_mm.tile([128, QCH], f32, tag="mm")
                    else:
                        bc_t = ps_bc.tile([128, QCH], f32, tag="bc")
                    ov_ap, bc_ap = ov_t[:], bc_t[:]
                rowsum_bcast(es, nkt, bc_ap)
                for kt in range(nkt):
                    nc.tensor.matmul(
                        ov_ap,
                        v_sb[:, kt, :],
                        es[:, kt, :],
                        start=(kt == 0),
                        stop=(kt == nkt - 1),
                    )
                rcp = rcppool.tile([128, QCH], f32)
                nc.vector.reciprocal_approx_fast(rcp[:], bc_ap)
                nc.vector.tensor_mul(at_sb[:, h, q0 : q0 + QCH], ov_ap, rcp[:])

            for _rep in range(reps):
                for b in range(B):
                    qt_sb = qkvpool.tile([128, NHL, S], bf16, tag="qt")
                    kt_sb = qkvpool.tile([128, S], bf16, tag="kt")
                    v_sb = qkvpool.tile([128, NKT, 128], bf16, tag="v")
                    at_sb = atpool.tile([128, NHL, S], bf16, tag="at")

                    for c in range(NCH):
                        xt_sb = xpool.tile([128, NDT, TOKCH], bf16)
                        proj_chunk(b, c, xt_sb, qt_sb, kt_sb, v_sb)
                        emit_wo(4)

                    for qc in range(NQC):
                        for h in range(NHL):
                            attn_unit(b, qc, h, qt_sb, kt_sb, v_sb, at_sb)
                            emit_wo(8)
                        for ht in range(HID // 128):
                            wo_jobs.append((b, at_sb, qc * QCH, ht))
            emit_wo(len(wo_jobs))

    nc.compile()
    return nc


def _get_nc(reps=1, cfg=None):
    key = ("nc", reps, tuple(sorted((cfg or {}).items())))
    if key not in _CACHED:
        _CACHED[key] = _build(reps, cfg)
    return _CACHED[key]


def _host_masks():
    kk = np.arange(128)[:, None]
    qq = np.arange(QCH)[None, :]
    m = np.empty((128, 4, QCH), dtype=BF)
    for j in range(4):
        m[:, j, :] = (kk <= qq - 128 * j).astype(BF)
    return m


def make_in_maps(hidden_states, Wq, Wk, Wv, Wo):
    hidden_states = np.asarray(hidden_states)
    Wq, Wk, Wv, Wo = (np.asarray(w) for w in (Wq, Wk, Wv, Wo))
    xt = np.ascontiguousarray(
        hidden_states.astype(BF).transpose(0, 2, 1)
    )  # [B, HID, S]
    msk = _host_masks()
    in_maps = []
    for c in range(8):
        in_maps.append(
            {
                "xt": xt,
                "wq": np.ascontiguousarray(Wq[:, c * CW : (c + 1) * CW]).astype(BF),
                "wk": np.ascontiguousarray(Wk[:, c * D : (c + 1) * D]).astype(BF),
                "wv": np.ascontiguousarray(Wv[:, c * D : (c + 1) * D]).astype(BF),
                "wo": np.ascontiguousarray(Wo[c * CW : (c + 1) * CW, :]).astype(BF),
                "msk": msk,
            }
        )
    return in_maps


def kernel(hidden_states, Wq, Wk, Wv, Wo):
    from concourse.bass_utils import run_bass_kernel_spmd

    nc = _get_nc()
    in_maps = make_in_maps(hidden_states, Wq, Wk, Wv, Wo)
    res = run_bass_kernel_spmd(nc, in_maps, core_ids=list(range(8)))
    _CACHED["last_results"] = res

    acc = res.results[0]["out_t"].astype(np.float32, copy=True)
    for c in range(1, 8):
        acc += res.results[c]["out_t"]
    out = np.ascontiguousarray(acc.transpose(0, 2, 1))  # [B, S, HID]
    return out



# revision 2
# speedup vs baseline: 1.1222x; 1.1222x over previous
"""GQA multi-head attention (B=2, S=2048, HID=4096, 32 q-heads / 8 kv-heads,
tanh soft-cap, causal) on 8 TRN2 NeuronCores.

Sharding: tensor-parallel over heads. Core c owns kv-head c and q-heads
4c..4c+3 (Wq/Wk/Wv column slices, Wo row slice). Each core computes a partial
output out_c^T; the host sums the 8 partials and transposes back.

Layout strategy on-core: all activations kept transposed (feature-major:
partition = feature, free = token).
  QT[d, t] = Wq^T X^T      (moving operand = X^T chunks, stationary = Wq tiles)
  KT[d, t] = Wk^T X^T
  V [t, d]                 (via PE transpose of the VT projection)
  S^T[k, q] = KT_tile-as-stationary @ QT            (one matmul per k-tile)
  P^T = exp(30*tanh(S^T * mult/30)) * causal_mask   (ScalarE; capped scores
                                                     need no max subtraction)
  rowsum: DVE adds es tiles -> acc, ONE ones^T @ acc matmul broadcasts
  O'^T[d, q] = V_tile-as-stationary @ P^T           (accumulated over k-tiles)
  A^T = O'^T * 1/rowsum                             (DVE, evict to bf16)
  out^T[hid, t] = Wo_tile-as-stationary @ A^T       (partial, bf16 to HBM)

v2 changes over the first working version (939.9us):
 - rowsum moved off the PE (DVE tile-adds + single broadcast matmul) —
   the per-k-tile ones-matmuls were ~63us of pure-overhead PE time.
 - causal diagonal trim: scores/PV matmuls, activations, masks and adds
   only touch the q >= k wedge of diagonal 128-k-tiles (partial-width
   APs; the skipped region is never read).
 - startup: xt chunk 0 + wk/wv/wq loads issued first (wq split so the
   first projection chain can start after 1MB); wo load deferred past
   chunk 1. First matmul ~3us instead of 51us (all weight DMAs used to
   serialize on the sync sequencer ahead of it).
 - PSUM re-plan: scores pool doubles as projection-chain pool (proj
   phase and attention phase are disjoint), freeing a bank so the Wo
   drain gets 2 PSUM bufs (kills the 0.8us/job eviction stall chain).
 - per-unit PE interleave: scores pair p, then PV of pair p-1, with one
   deferred Wo job as filler, so the PE never waits on the scalar
   engine's tanh+exp drip.
 - last query-block of the last batch split into two 256-wide windows
   so the final Wo drain is half as deep.
 - projections use 256-wide token chunks (half the matmul dispatches).
"""

import sys

if "/opt/trn_rl_repo" not in sys.path:
    sys.path.insert(0, "/opt/trn_rl_repo")

import numpy as np
import ml_dtypes

BF = ml_dtypes.bfloat16

HID = 4096
S = 2048
B = 2
D = 128          # head dim
NHL = 4          # local q heads per core
CW = NHL * D     # 512, local q-proj width / wo row count
TOKCH = 256      # token chunk for projections
NCH = S // TOKCH
QCH = 512        # max query window for attention
NKT = S // 128   # k-tiles per batch
NDT = HID // 128
ATTN_MULT = 0.08838834764831845
CAP = 30.0

_CACHED = {}

DEF_CFG = dict(
    sc_bufs=2, wo_bufs=2, tanh=True, trim=True, split_last=True,
)


def _build(reps=1, cfg=None):
    cfg = dict(DEF_CFG, **(cfg or {}))
    import concourse.mybir as mybir
    import concourse.tile as tile
    from concourse import bacc
    from concourse.masks import make_identity

    bf16 = mybir.dt.bfloat16
    f32 = mybir.dt.float32

    nc = bacc.Bacc(num_devices=8)
    xt_d = nc.dram_tensor("xt", [B, HID, S], bf16, kind="ExternalInput")
    wq_d = nc.dram_tensor("wq", [HID, CW], bf16, kind="ExternalInput")
    wk_d = nc.dram_tensor("wk", [HID, D], bf16, kind="ExternalInput")
    wv_d = nc.dram_tensor("wv", [HID, D], bf16, kind="ExternalInput")
    wo_d = nc.dram_tensor("wo", [CW, HID], bf16, kind="ExternalInput")
    msk_d = nc.dram_tensor("msk", [128, 4, QCH], bf16, kind="ExternalInput")
    out_d = nc.dram_tensor("out_t", [B, HID, S], bf16, kind="ExternalOutput")

    Tanh = mybir.ActivationFunctionType.Tanh
    Exp = mybir.ActivationFunctionType.Exp

    with tile.TileContext(nc) as tc:
        with (
            tc.tile_pool(name="consts", bufs=1) as consts,
            tc.tile_pool(name="weights", bufs=1) as wpool,
            tc.tile_pool(name="xin", bufs=2) as xpool,
            tc.tile_pool(name="qkv", bufs=1) as qkvpool,
            tc.tile_pool(name="atp", bufs=1) as atpool,
            tc.tile_pool(name="es", bufs=2) as espool,
            tc.tile_pool(name="rcp", bufs=2) as rcppool,
            tc.tile_pool(name="accp", bufs=2) as accpool,
            tc.tile_pool(name="vst", bufs=2) as vstpool,
            tc.tile_pool(name="osta", bufs=3) as outpool,
            # PSUM: sc 2x2 banks (scores pairs; proj chains reuse this
            # ring — the phases are disjoint), ov 1, mm 1 (rowsum bcast
            # + v-transpose), wo 2.  Total 8 banks.
            tc.tile_pool(name="ps_sc", bufs=cfg["sc_bufs"], space="PSUM") as ps_sc,
            tc.tile_pool(name="ps_ov", bufs=1, space="PSUM") as ps_ov,
            tc.tile_pool(name="ps_mm", bufs=1, space="PSUM") as ps_mm,
            tc.tile_pool(name="ps_wo", bufs=cfg["wo_bufs"], space="PSUM") as ps_wo,
        ):
            # --- persistent weights/constants in SBUF ---
            wq_sb = wpool.tile([128, NDT, CW], bf16, tag="wq")
            wk_sb = wpool.tile([128, NDT, D], bf16, tag="wk")
            wv_sb = wpool.tile([128, NDT, D], bf16, tag="wv")
            wo_sb = wpool.tile([128, CW // 128, HID], bf16, tag="wo")
            msk_sb = consts.tile([128, 4, QCH], bf16)

            xt_r = xt_d.ap().rearrange("b (po pi) t -> pi b po t", pi=128)
            wq_r = wq_d.ap().rearrange("(po pi) f -> pi po f", pi=128)
            wk_r = wk_d.ap().rearrange("(po pi) f -> pi po f", pi=128)
            wv_r = wv_d.ap().rearrange("(po pi) f -> pi po f", pi=128)
            wo_r = wo_d.ap().rearrange("(po pi) f -> pi po f", pi=128)

            # first xt chunk's doorbell goes before the weight loads so
            # its (parallel-queue) transfer overlaps the weight DIRECT2Ds
            xt0_sb = xpool.tile([128, NDT, TOKCH], bf16, tag="xt")
            nc.sync.dma_start(xt0_sb[:], xt_r[:, 0, :, 0:TOKCH])
            nc.sync.dma_start(wk_sb[:], wk_r)
            nc.sync.dma_start(wv_sb[:], wv_r)
            for po0 in range(0, NDT, 8):
                nc.sync.dma_start(
                    wq_sb[:, po0 : po0 + 8, :], wq_r[:, po0 : po0 + 8, :]
                )
            nc.sync.dma_start(msk_sb[:], msk_d.ap())

            ones_f32 = consts.tile([128, 128], f32)
            nc.vector.memset(ones_f32[:], 1.0)
            ident = consts.tile([128, 128], bf16)
            make_identity(nc, ident[:])

            wo_jobs = []

            def drain(n, engine="vector"):
                # drain up to n deferred Wo output-tile jobs; interleaved
                # among attention/projection work as PE filler
                for _ in range(min(n, len(wo_jobs))):
                    jb, jat, jq0, jqlen, ht = wo_jobs.pop(0)
                    po = ps_wo.tile([128, QCH], f32, tag="wo")
                    for ct in range(CW // 128):
                        nc.tensor.matmul(
                            po[:, :jqlen],
                            wo_sb[:, ct, ht * 128 : (ht + 1) * 128],
                            jat[:, ct, jq0 : jq0 + jqlen],
                            start=(ct == 0),
                            stop=(ct == CW // 128 - 1),
                        )
                    ost = outpool.tile([128, QCH], bf16)
                    if engine == "scalar":
                        nc.scalar.copy(ost[:, :jqlen], po[:, :jqlen])
                    else:
                        nc.vector.tensor_copy(ost[:, :jqlen], po[:, :jqlen])
                    nc.sync.dma_start(
                        out_d.ap()[jb, ht * 128 : (ht + 1) * 128, jq0 : jq0 + jqlen],
                        ost[:, :jqlen],
                    )

            def chain(w_sb, hsl, xt_sb, out_ap):
                p = ps_sc.tile([128, TOKCH], f32, tag="sc")
                for dt in range(NDT):
                    nc.tensor.matmul(
                        p[:],
                        w_sb[:, dt, hsl],
                        xt_sb[:, dt, :],
                        start=(dt == 0),
                        stop=(dt == NDT - 1),
                    )
                nc.vector.tensor_copy(out_ap, p[:])

            def proj_chunk(b, c, qt_sb, kt_sb, v_sb, xt_pre=None):
                t0 = c * TOKCH
                if xt_pre is None:
                    xt_sb = xpool.tile([128, NDT, TOKCH], bf16, tag="xt")
                    nc.sync.dma_start(xt_sb[:], xt_r[:, b, :, t0 : t0 + TOKCH])
                else:
                    xt_sb = xt_pre
                # K/V first (1MB weights each — ready earliest at startup)
                chain(wk_sb, slice(0, D), xt_sb, kt_sb[:, t0 : t0 + TOKCH])
                vt_sb = vstpool.tile([128, TOKCH], bf16)
                chain(wv_sb, slice(0, D), xt_sb, vt_sb[:])
                for i in range(TOKCH // 128):
                    tt = c * (TOKCH // 128) + i
                    pool, tag = ((ps_mm, "mm"), (ps_wo, "wo"))[i % 2]
                    ptp = pool.tile([128, 128], bf16, tag=tag)
                    nc.tensor.transpose(
                        ptp[:], vt_sb[:, i * 128 : (i + 1) * 128], ident[:]
                    )
                    nc.vector.tensor_copy(v_sb[:, tt, :], ptp[:])
                for h in range(NHL):
                    chain(
                        wq_sb, slice(h * 128, (h + 1) * 128), xt_sb,
                        qt_sb[:, h, t0 : t0 + TOKCH],
                    )
                drain(4, engine="scalar")

            def attn_unit(b, h, q0, qlen, qt_sb, kt_sb, v_sb, at_sb):
                t0 = q0 // 128
                nkt = t0 + qlen // 128
                npair = (nkt + 1) // 2
                trim = cfg["trim"]
                es = espool.tile([128, NKT, QCH], bf16, tag="es")
                acc = accpool.tile([128, QCH], f32, tag="acc")
                ov = ps_ov.tile([128, QCH], f32, tag="ov")
                done_pv = [0]

                def off_of(kt):
                    return max(0, 128 * (kt - t0)) if trim else 0

                def pv_upto(n):
                    for kt in range(done_pv[0], n):
                        off = off_of(kt)
                        nc.tensor.matmul(
                            ov[:, off:qlen],
                            v_sb[:, kt, :],
                            es[:, kt, off:qlen],
                            start=(kt == 0),
                            stop=(kt == nkt - 1),
                            skip_group_check=True,
                        )
                    done_pv[0] = max(done_pv[0], n)

                for p in range(npair):
                    kts = [kt for kt in (2 * p, 2 * p + 1) if kt < nkt]
                    ps = ps_sc.tile([128, 2, QCH], f32, tag="sc")
                    offs = [off_of(kt) for kt in kts]
                    for i, kt in enumerate(kts):
                        off = offs[i]
                        nc.tensor.matmul(
                            ps[:, i, off:qlen],
                            kt_sb[:, kt * 128 : (kt + 1) * 128],
                            qt_sb[:, h, q0 + off : q0 + qlen],
                            start=True,
                            stop=True,
                        )
                    if len(kts) == 2 and offs[0] == 0 and offs[1] == 0 and qlen == QCH:
                        if cfg["tanh"]:
                            nc.scalar.activation(
                                ps[:], ps[:], Tanh, scale=ATTN_MULT / CAP
                            )
                            nc.scalar.activation(
                                es[:, 2 * p : 2 * p + 2, :], ps[:], Exp, scale=CAP
                            )
                        else:
                            nc.scalar.activation(
                                es[:, 2 * p : 2 * p + 2, :], ps[:], Exp,
                                scale=ATTN_MULT,
                            )
                    else:
                        for i, kt in enumerate(kts):
                            off = offs[i]
                            if cfg["tanh"]:
                                nc.scalar.activation(
                                    ps[:, i, off:qlen], ps[:, i, off:qlen], Tanh,
                                    scale=ATTN_MULT / CAP,
                                )
                                nc.scalar.activation(
                                    es[:, kt, off:qlen], ps[:, i, off:qlen], Exp,
                                    scale=CAP,
                                )
                            else:
                                nc.scalar.activation(
                                    es[:, kt, off:qlen], ps[:, i, off:qlen], Exp,
                                    scale=ATTN_MULT,
                                )
                    # DVE: mask the diagonal wedge, then fold into rowsum acc
                    for i, kt in enumerate(kts):
                        off = offs[i]
                        if kt >= t0:
                            nc.vector.tensor_mul(
                                es[:, kt, off:qlen],
                                es[:, kt, off:qlen],
                                msk_sb[:, kt - t0, off:qlen],
                            )
                        if kt == 0:
                            nc.vector.tensor_copy(acc[:, :qlen], es[:, 0, :qlen])
                        else:
                            nc.vector.tensor_add(
                                acc[:, off:qlen],
                                acc[:, off:qlen],
                                es[:, kt, off:qlen],
                            )
                    if p >= 1:
                        pv_upto(2 * p)
                    drain(1)
                pv_upto(nkt)
                bc = ps_mm.tile([128, QCH], f32, tag="mm")
                nc.tensor.matmul(
                    bc[:, :qlen], ones_f32[:], acc[:, :qlen], start=True, stop=True
                )
                rcp = rcppool.tile([128, QCH], f32)
                nc.vector.reciprocal_approx_fast(rcp[:, :qlen], bc[:, :qlen])
                nc.vector.tensor_mul(
                    at_sb[:, h, q0 : q0 + qlen], ov[:, :qlen], rcp[:, :qlen]
                )
                drain(2)

            for _rep in range(reps):
                for b in range(B):
                    qt_sb = qkvpool.tile([128, NHL, S], bf16, tag="qt")
                    kt_sb = qkvpool.tile([128, S], bf16, tag="kt")
                    v_sb = qkvpool.tile([128, NKT, 128], bf16, tag="v")
                    at_sb = atpool.tile([128, NHL, S], bf16, tag="at")

                    for c in range(NCH):
                        xt_pre = xt0_sb if (b == 0 and c == 0 and _rep == 0) else None
                        proj_chunk(b, c, qt_sb, kt_sb, v_sb, xt_pre=xt_pre)
                        if b == 0 and c == 1 and _rep == 0:
                            # wo load deferred here: its DIRECT2D no longer
                            # gates the first projection matmuls
                            nc.sync.dma_start(wo_sb[:], wo_r)

                    wins = [(q * QCH, QCH) for q in range(S // QCH)]
                    if cfg["split_last"] and b == B - 1:
                        wins = wins[:-1] + [(S - QCH, QCH // 2), (S - QCH // 2, QCH // 2)]
                    for q0, qlen in wins:
                        for h in range(NHL):
                            attn_unit(b, h, q0, qlen, qt_sb, kt_sb, v_sb, at_sb)
                        for ht in range(HID // 128):
                            wo_jobs.append((b, at_sb, q0, qlen, ht))
            drain(len(wo_jobs))

    nc.compile()
    return nc


def _get_nc(reps=1, cfg=None):
    key = ("nc", reps, tuple(sorted((cfg or {}).items())))
    if key not in _CACHED:
        _CACHED[key] = _build(reps, cfg)
    return _CACHED[key]


def _host_masks():
    kk = np.arange(128)[:, None]
    qq = np.arange(QCH)[None, :]
    m = np.empty((128, 4, QCH), dtype=BF)
    for j in range(4):
        m[:, j, :] = (kk <= qq - 128 * j).astype(BF)
    return m


def make_in_maps(hidden_states, Wq, Wk, Wv, Wo):
    hidden_states = np.asarray(hidden_states)
    Wq, Wk, Wv, Wo = (np.asarray(w) for w in (Wq, Wk, Wv, Wo))
    xt = np.ascontiguousarray(
        hidden_states.astype(BF).transpose(0, 2, 1)
    )  # [B, HID, S]
    msk = _host_masks()
    in_maps = []
    for c in range(8):
        in_maps.append(
            {
                "xt": xt,
                "wq": np.ascontiguousarray(Wq[:, c * CW : (c + 1) * CW]).astype(BF),
                "wk": np.ascontiguousarray(Wk[:, c * D : (c + 1) * D]).astype(BF),
                "wv": np.ascontiguousarray(Wv[:, c * D : (c + 1) * D]).astype(BF),
                "wo": np.ascontiguousarray(Wo[c * CW : (c + 1) * CW, :]).astype(BF),
                "msk": msk,
            }
        )
    return in_maps


def kernel(hidden_states, Wq, Wk, Wv, Wo):
    from concourse.bass_utils import run_bass_kernel_spmd

    nc = _get_nc()
    in_maps = make_in_maps(hidden_states, Wq, Wk, Wv, Wo)
    res = run_bass_kernel_spmd(nc, in_maps, core_ids=list(range(8)))
    _CACHED["last_results"] = res

    acc = res.results[0]["out_t"].astype(np.float32, copy=True)
    for c in range(1, 8):
        acc += res.results[c]["out_t"]
    out = np.ascontiguousarray(acc.transpose(0, 2, 1))  # [B, S, HID]
    return out
